# revision 3
# baseline (speedup 1.0000x reference)
"""Trainium2 Bass kernel for nn_Ensemble (dense MLP ensemble, E=8, B=65536).

Network (per ensemble member e):
    x   = concat(inputs[..., :48], clip(inputs[..., 48:64], -1, 1))   # [B, 64]
    h1  = relu(x @ W1[e] + b1[e])                                     # [B, 128]
    h2  = relu(h1 @ W2[e] + b2[e])                                    # [B, 128]
    out = h2 @ W3[e] + b3[e]                                          # [B, 48]

Sharding: ensemble dim E=8 across the 8 NeuronCores (one member per core,
weights core-resident).  Feature-major layout: host packs x.T into
X = [128, B/2] bf16 (rows 0:64 = features batch-half 0, rows 64:128 =
batch-half 1), so all layers are weight-stationary matmuls streaming batch.

Key TRN2 constraint: matmul PSUM output is fp32, so every PSUM->SBUF drain
(relu+bias+cast) runs at 1 elem/cycle/lane on ScalarE (1.2 GHz) or VectorE
(0.96 GHz).  Total drain columns/core = (128+128+64)*65536/128 = 163840,
so the drains — not the PE — are the roofline (~76us combined).  The design
keeps both drain engines saturated:

  - Quantum = 512 x-cols (1024 samples).  Per quantum: L1 as a row-tiled
    K=64 MM pair -> [128,1024] PSUM tile; drain; L2 as 2 dense MMs ->
    [128,1024]; drain; per quantum pair: L3 as 4 col-tiled M=64 MMs ->
    [128,1024] (out for 2048 samples); drain.
  - One shared PSUM pool, 4 bufs x 2 banks: steady state = ACT draining one
    tile, DVE another, PE filling the other two.  Drains are greedily
    load-balanced between ScalarE and VectorE by model cost (1127/1332 ns).
  - DMA: 2 MB x/out chunks, double-buffered.
"""

import numpy as np
import ml_dtypes

BF16 = ml_dtypes.bfloat16

E = 8
B = 65536
HB = B // 2          # batch half (free-dim columns per core)
IN = 64
AC = 16              # clipped action features (last 16)
H = 128
OUT = 48
OUTP = 64            # padded out features (col-group alignment)

CHUNK = 8192         # free-dim columns per x/out DMA chunk
NT = 512             # matmul free dim (one PSUM bank of fp32)
QX = 512             # x-cols per quantum (-> 1024 psum cols per layer tile)

_CACHED = None

# model costs (ns) for a [128,1024] fp32 PSUM->SBUF drain per engine
_ACT_DRAIN_NS = 1127.0
_DVE_DRAIN_NS = 1332.0


def _build_nc(reps=None):
    """Build the bass module. reps=None -> plain kernel; reps=R wraps the
    body in a hardware For_i loop (self-timing variant)."""
    import contextlib
    import concourse.bacc as bacc
    import concourse.mybir as mybir
    import concourse.tile as tile

    f32 = mybir.dt.float32
    bf16 = mybir.dt.bfloat16
    AF = mybir.ActivationFunctionType
    ALU = mybir.AluOpType

    nc = bacc.Bacc("TRN2", target_bir_lowering=False)

    x_d = nc.dram_tensor("x", [128, HB], bf16, kind="ExternalInput")
    w1_d = nc.dram_tensor("w1p", [128, H], bf16, kind="ExternalInput")
    w2_d = nc.dram_tensor("w2", [H, H], bf16, kind="ExternalInput")
    w3_d = nc.dram_tensor("w3p", [H, OUTP], bf16, kind="ExternalInput")
    b1_d = nc.dram_tensor("b1v", [H, 1], f32, kind="ExternalInput")
    b2_d = nc.dram_tensor("b2v", [H, 1], f32, kind="ExternalInput")
    b3_d = nc.dram_tensor("b3v", [128, 1], f32, kind="ExternalInput")
    out_d = nc.dram_tensor("out", [128, HB], bf16, kind="ExternalOutput")

    with tile.TileContext(nc) as tc:
        with (
            tc.tile_pool(name="consts", bufs=1) as consts,
            tc.tile_pool(name="xp", bufs=2) as xp,
            tc.tile_pool(name="h1sb", bufs=4) as h1pool,
            tc.tile_pool(name="h2sb", bufs=4) as h2pool,
            tc.tile_pool(name="osb", bufs=2) as opool,
            tc.tile_pool(name="ps", bufs=4, space="PSUM") as ps,
        ):
            w1_sb = consts.tile([128, H], bf16)
            w2_sb = consts.tile([H, H], bf16)
            w3_sb = consts.tile([H, OUTP], bf16)
            b1_sb = consts.tile([H, 1], f32)
            b2_sb = consts.tile([H, 1], f32)
            b3_sb = consts.tile([128, 1], f32)
            nc.sync.dma_start(out=w1_sb, in_=w1_d[:])
            nc.sync.dma_start(out=w2_sb, in_=w2_d[:])
            nc.sync.dma_start(out=w3_sb, in_=w3_d[:])
            nc.sync.dma_start(out=b1_sb, in_=b1_d[:])
            nc.sync.dma_start(out=b2_sb, in_=b2_d[:])
            nc.sync.dma_start(out=b3_sb, in_=b3_d[:])

            # greedy ACT/DVE balance across all drains
            eng_t = {"act": 0.0, "dve": 0.0}

            def drain(dst, src, bias_sb, relu):
                if eng_t["act"] + _ACT_DRAIN_NS <= eng_t["dve"] + _DVE_DRAIN_NS:
                    eng_t["act"] += _ACT_DRAIN_NS
                    nc.scalar.activation(
                        dst, src, AF.Relu if relu else AF.Identity, bias=bias_sb)
                else:
                    eng_t["dve"] += _DVE_DRAIN_NS
                    if relu:
                        nc.vector.tensor_scalar(dst, src, bias_sb, 0.0,
                                                op0=ALU.add, op1=ALU.max)
                    else:
                        nc.vector.tensor_scalar(dst, src, bias_sb, None,
                                                op0=ALU.add)

            loop = (tc.For_i(0, reps, 1, hint_engines=(mybir.EngineType.PE,))
                    if reps is not None else contextlib.nullcontext())
            with loop:
                for c in range(HB // CHUNK):
                    x_t = xp.tile([128, CHUNK], bf16)
                    nc.sync.dma_start(out=x_t,
                                      in_=x_d[:, c * CHUNK:(c + 1) * CHUNK])
                    o_t = opool.tile([128, CHUNK], bf16)
                    h2_prev = None
                    for q in range(CHUNK // QX):
                        xc = q * QX
                        # L1: row-tiled K=64 pair -> [128, 1024] fp32
                        h1ps = ps.tile([128, 2 * NT], f32, tag="mm")
                        nc.tensor.matmul(h1ps[:, 0:NT], w1_sb[0:64, :],
                                         x_t[0:64, xc:xc + QX],
                                         start=True, stop=True)
                        nc.tensor.matmul(h1ps[:, NT:2 * NT], w1_sb[64:128, :],
                                         x_t[64:128, xc:xc + QX],
                                         start=True, stop=True)
                        h1sb = h1pool.tile([128, 2 * NT], bf16)
                        drain(h1sb, h1ps, b1_sb, relu=True)
                        # L2: dense K=128 -> [128, 1024] fp32
                        h2ps = ps.tile([128, 2 * NT], f32, tag="mm")
                        nc.tensor.matmul(h2ps[:, 0:NT], w2_sb,
                                         h1sb[:, 0:NT], start=True, stop=True)
                        nc.tensor.matmul(h2ps[:, NT:2 * NT], w2_sb,
                                         h1sb[:, NT:2 * NT],
                                         start=True, stop=True)
                        h2sb = h2pool.tile([128, 2 * NT], bf16)
                        drain(h2sb, h2ps, b2_sb, relu=True)
                        if q % 2 == 0:
                            h2_prev = h2sb
                            continue
                        # L3 over the quantum pair: col-tiled M=64, two
                        # batch tiles share each psum column range
                        ops_t = ps.tile([128, 2 * NT], f32, tag="mm")
                        for j, hsb in ((0, h2_prev), (1, h2sb)):
                            nc.tensor.matmul(
                                ops_t[0:OUTP, j * NT:(j + 1) * NT], w3_sb,
                                hsb[:, 0:NT],
                                start=True, stop=True, tile_position=(0, 0))
                            nc.tensor.matmul(
                                ops_t[OUTP:128, j * NT:(j + 1) * NT], w3_sb,
                                hsb[:, NT:2 * NT],
                                start=True, stop=True, tile_position=(0, OUTP))
                        oc = (q // 2) * 2 * NT
                        drain(o_t[:, oc:oc + 2 * NT], ops_t, b3_sb, relu=False)
                    nc.sync.dma_start(out=out_d[:, c * CHUNK:(c + 1) * CHUNK],
                                      in_=o_t)

    nc.compile()
    return nc


def _get_nc():
    global _CACHED
    if _CACHED is None:
        _CACHED = _build_nc()
    return _CACHED


def _prep_member(x_e, W1_e, b1_e, W2_e, b2_e, W3_e, b3_e):
    """Host-side shard prep: transpose to feature-major, pack the two batch
    halves on the partition axis, clip action features, cast to bf16."""
    xt = np.ascontiguousarray(np.asarray(x_e).T)      # [64, B] f32
    np.clip(xt[IN - AC:IN], -1.0, 1.0, out=xt[IN - AC:IN])
    X = np.empty((128, HB), dtype=BF16)
    X[0:64] = xt[:, :HB]
    X[64:128] = xt[:, HB:]

    w1p = np.empty((128, H), dtype=BF16)
    w1p[0:64] = W1_e
    w1p[64:128] = W1_e
    w2 = W2_e.astype(BF16)
    w3p = np.zeros((H, OUTP), dtype=BF16)
    w3p[:, :OUT] = W3_e
    b1v = np.ascontiguousarray(b1_e.astype(np.float32).reshape(H, 1))
    b2v = np.ascontiguousarray(b2_e.astype(np.float32).reshape(H, 1))
    b3v = np.zeros((128, 1), dtype=np.float32)
    b3v[0:OUT, 0] = b3_e
    b3v[OUTP:OUTP + OUT, 0] = b3_e
    return {"x": X, "w1p": w1p, "w2": w2, "w3p": w3p,
            "b1v": b1v, "b2v": b2v, "b3v": b3v}


def kernel(**inputs):
    from concourse.bass_utils import run_bass_kernel_spmd

    x = np.asarray(inputs["inputs"], dtype=np.float32).reshape(E, B, IN)
    W1 = np.asarray(inputs["W1"], dtype=np.float32)
    b1 = np.asarray(inputs["b1"], dtype=np.float32)
    W2 = np.asarray(inputs["W2"], dtype=np.float32)
    b2 = np.asarray(inputs["b2"], dtype=np.float32)
    W3 = np.asarray(inputs["W3"], dtype=np.float32)
    b3 = np.asarray(inputs["b3"], dtype=np.float32)

    in_maps = [
        _prep_member(x[e], W1[e], b1[e], W2[e], b2[e], W3[e], b3[e])
        for e in range(E)
    ]

    nc = _get_nc()
    res = run_bass_kernel_spmd(nc, in_maps, core_ids=list(range(E)))

    out = np.empty((E, B, OUT), dtype=np.float32)
    for e in range(E):
        dev = res.results[e]["out"]          # [128, HB] bf16
        out[e, :HB] = dev[0:OUT, :].T
        out[e, HB:] = dev[OUTP:OUTP + OUT, :].T
    return out
